# revision 1
# baseline (speedup 1.0000x reference)
"""Causal multi-head attention (B=4, T=2048, D=1024, H=16) on 8 Trainium2 cores.

Sharding (data + tensor parallel): core c handles batch b = c//2 and head-group
g = c%2 (8 of the 16 heads). Wq/Wk/Wv are column-sharded by head, Wp is
row-sharded; the two per-batch partial outputs are summed on the host (this
replaces the device all-reduce — the host-side sum is the unshard step).

Per-core kernel (all matmuls in float32r = TF32-like, full PE rate at N>=256):
  - everything is computed in "transposed space" to avoid on-chip transposes:
      Qt/Kt [head_dims, t] = W_slice @ x.T       (x passed pre-transposed)
      V     [t, head_dims] = x @ Wv_slice.T      (natural layout)
      St    = scores.T tile blocks [k, q] = Kt_tile.T-contracted with Qt
      E     = exp(St)  (1/sqrt(64) pre-folded into Wq; causal mask applied as
              a 0/1 multiply on the diagonal 128x128 zones; k-tiles fully
              above the diagonal are skipped, diagonal tiles are narrowed)
      ctxT_aug [65, q] = Vaug.T @ E  accumulated over k-tiles in PSUM, where
              Vaug carries a ones-column so row 64 is the softmax denominator
      ctx   = ctxT_aug[0:64] * broadcast(1/denominator)   (PE ones-broadcast)
      outT  = WpT_slice.T @ ctx  -> partial output, summed on host
  - St tiles are computed into PSUM bank-pairs [128, 1024] so one ACT exp
    covers two k-tiles (halves the ACT per-op overhead); the two heads of a
    head-pair are interleaved to hide exp latency behind PE work.
"""
import numpy as np

T = 2048
D = 1024
B = 4
H = 16
HL = 8            # heads per core
NP = 4            # head pairs per core
QB = 512          # q-block width (one PSUM bank of fp32)
NQB = T // QB
NKT = T // 128

_COMPILED = None


# --------------------------------------------------------------------------
# bass kernel build
# --------------------------------------------------------------------------
def _build_bass():
    import concourse.bass as bass
    import concourse.mybir as mybir
    from concourse.tile import TileContext

    F32 = mybir.dt.float32
    F32R = mybir.dt.float32r
    Act = mybir.ActivationFunctionType
    Alu = mybir.AluOpType

    nc = bass.Bass()
    xt = nc.dram_tensor("xt", [D, T], F32R, kind="ExternalInput")
    wq = nc.dram_tensor("wq", [D, 512], F32R, kind="ExternalInput")
    wk = nc.dram_tensor("wk", [D, 512], F32R, kind="ExternalInput")
    wv = nc.dram_tensor("wv", [D, 512], F32R, kind="ExternalInput")
    wp = nc.dram_tensor("wp", [512, D], F32R, kind="ExternalInput")
    mask1 = nc.dram_tensor("mask1", [128, 128], F32R, kind="ExternalInput")
    ones64 = nc.dram_tensor("ones64", [1, 64], F32R, kind="ExternalInput")
    vones = nc.dram_tensor("vones", [128, NKT * HL], F32R, kind="ExternalInput")
    outt = nc.dram_tensor("outt", [D, T], F32, kind="ExternalOutput")

    with TileContext(nc) as tc, nc.allow_low_precision(reason="f32r pipeline"):
        with tc.tile_pool(name="wts", bufs=1) as wts, \
             tc.tile_pool(name="xp", bufs=2) as xp, \
             tc.tile_pool(name="big", bufs=1) as big, \
             tc.tile_pool(name="qtp", bufs=4) as qtp, \
             tc.tile_pool(name="ctp", bufs=4) as ctp, \
             tc.tile_pool(name="ep", bufs=4) as ep, \
             tc.tile_pool(name="sm", bufs=1) as sm, \
             tc.tile_pool(name="osb", bufs=4) as osb, \
             tc.tile_pool(name="pmm", bufs=3, space="PSUM") as pmm, \
             tc.tile_pool(name="pca", bufs=2, space="PSUM") as pca:

            # weights/constants; DMAs split per k-slice so compute starts as
            # soon as the first slices land
            wq_t = wts.tile([128, 8, 512], F32R, tag="wq")
            wk_t = wts.tile([128, 8, 512], F32R, tag="wk")
            wv_t = wts.tile([128, 8, 512], F32R, tag="wv")
            wp_t = wts.tile([128, 4, 1024], F32R, tag="wp")
            wqr = wq[:].rearrange("(n p) m -> p n m", p=128)
            wkr = wk[:].rearrange("(n p) m -> p n m", p=128)
            wvr = wv[:].rearrange("(n p) m -> p n m", p=128)
            x_tiles = {}

            def load_x(tb):
                x_t = xp.tile([128, 8, QB], F32R, tag="x", name=f"x_t{tb}a")
                xr = xt[:, QB * tb:QB * (tb + 1)].rearrange("(n p) m -> p n m", p=128)
                for kk in range(8):
                    nc.sync.dma_start(x_t[:, kk, :], xr[:, kk, :])
                x_tiles[tb] = x_t

            xr0 = xt[:, 0:QB].rearrange("(n p) m -> p n m", p=128)
            x_t0 = xp.tile([128, 8, QB], F32R, tag="x", name="x_t0")
            for kk in range(8):
                nc.sync.dma_start(wq_t[:, kk, :], wqr[:, kk, :])
                nc.sync.dma_start(x_t0[:, kk, :], xr0[:, kk, :])
            x_tiles[0] = x_t0
            for kk in range(8):
                nc.sync.dma_start(wk_t[:, kk, :], wkr[:, kk, :])
            for kk in range(8):
                nc.sync.dma_start(wv_t[:, kk, :], wvr[:, kk, :])
            m1 = sm.tile([128, 128], F32R, tag="m1")
            nc.sync.dma_start(m1[:], mask1[:])
            on = sm.tile([1, 64], F32R, tag="on")
            nc.sync.dma_start(on[:], ones64[:])

            kt_t = big.tile([128, NP, T], F32R, tag="kt")
            va_t = big.tile([128, NKT, HL, 65], F32R, tag="va")
            nc.sync.dma_start(
                va_t[:, :, :, 64:65].squeeze(3),
                vones[:].rearrange("p (n h) -> p n h", n=NKT))
            wpr = wp[:].rearrange("(n p) m -> p n m", p=128)
            for kk in range(4):
                nc.sync.dma_start(wp_t[:, kk, :], wpr[:, kk, :])

            ctx_tiles = {}

            for tb in range(NQB):
                x_t = x_tiles[tb]
                if tb + 1 < NQB:
                    load_x(tb + 1)

                # projections for this t-block
                qt_tiles = {}
                for p in range(NP):
                    ps_q = pmm.tile([128, 2 * QB], F32, tag="mm", name=f"psq{tb}_{p}")
                    for kk in range(8):
                        nc.tensor.matmul(ps_q[:, 0:QB], wq_t[:, kk, 128 * p:128 * (p + 1)],
                                         x_t[:, kk, :], start=(kk == 0), stop=(kk == 7))
                    q_tile = qtp.tile([128, QB], F32R, tag="qt", name=f"qt{tb}_{p}")
                    nc.vector.tensor_copy(q_tile[:], ps_q[:, 0:QB])
                    qt_tiles[p] = q_tile
                for p in range(NP):
                    ps_k = pmm.tile([128, 2 * QB], F32, tag="mm", name=f"psk{tb}_{p}")
                    for kk in range(8):
                        nc.tensor.matmul(ps_k[:, 0:QB], wk_t[:, kk, 128 * p:128 * (p + 1)],
                                         x_t[:, kk, :], start=(kk == 0), stop=(kk == 7))
                    nc.vector.tensor_copy(kt_t[:, p, QB * tb:QB * (tb + 1)], ps_k[:, 0:QB])
                for tt in range(4):
                    ps_v = pmm.tile([128, 2 * QB], F32, tag="mm", name=f"psv{tb}_{tt}")
                    for kk in range(8):
                        nc.tensor.matmul(ps_v[:, 0:QB], x_t[:, kk, 128 * tt:128 * (tt + 1)],
                                         wv_t[:, kk, :], start=(kk == 0), stop=(kk == 7))
                    nc.vector.tensor_copy(
                        va_t[:, 4 * tb + tt, :, 0:64],
                        ps_v[:, 0:QB].rearrange("p (h d) -> p h d", h=HL))

                # attention for q-block j = tb
                j = tb
                nkt_j = 4 * j + 4
                # normalizations are queued and emitted one beat later so the
                # PE's static schedule has ready matmuls ahead of the
                # reciprocal-dependent broadcast
                pending_norms = []

                def make_norm(p, s, ctxa, ctx_tile, j=j):
                    def f():
                        # copy raw ctx + reciprocal out of PSUM (frees the
                        # ctxa bank), then PE-broadcast the reciprocal across
                        # partitions and normalize
                        recip = sm.tile([1, QB], F32R, tag="recip", bufs=1,
                                        name=f"recip{j}_{p}_{s}")
                        nc.vector.reciprocal(recip[:], ctxa[s][64:65, :])
                        raw = sm.tile([64, QB], F32, tag="raw", bufs=2,
                                      name=f"raw{j}_{p}_{s}")
                        nc.vector.tensor_copy(raw[:], ctxa[s][0:64, :])
                        bcp = pca.tile([128, QB], F32, tag="ctxa", name=f"bcp{j}_{p}_{s}")
                        nc.tensor.matmul(bcp[0:64, :], on[:], recip[:], start=True, stop=True)
                        nc.vector.tensor_tensor(ctx_tile[64 * s:64 * s + 64, :],
                                                raw[:], bcp[0:64, :], op=Alu.mult)
                    return f

                for p in range(NP):
                    ctx_tile = ctp.tile([128, QB], F32R, tag="ctx", name=f"ctx{j}_{p}")
                    ctx_tiles[(p, j)] = ctx_tile
                    q_tile = qt_tiles[p]
                    ctxa = [pca.tile([65, QB], F32, tag="ctxa", name=f"ctxa_{j}_{p}_{s2}")
                            for s2 in range(2)]

                    def do_norm(s, p=p, ctxa=ctxa, ctx_tile=ctx_tile):
                        pending_norms.append(make_norm(p, s, ctxa, ctx_tile))

                    for s in range(2):
                        if s == 1:
                            do_norm(0)
                        # descending k-tiles: diagonal (masked, narrowed) tiles
                        # first; partial-range start=True is sound because later
                        # full tiles overwrite where has_written is unset
                        for ip in reversed(range(nkt_j // 2)):
                            i0, i1 = 2 * ip, 2 * ip + 1
                            o0, o1 = i0 - 4 * j, i1 - 4 * j
                            cs0 = 0 if o0 < 0 else min(128 * o0, 256)
                            cs1 = 0 if o1 < 0 else min(128 * o1, 256)
                            h = 2 * p + s
                            hs = slice(64 * s, 64 * s + 64)
                            tp = (64 * s, 0)
                            st2 = pmm.tile([128, 2 * QB], F32, tag="mm",
                                           name=f"st{j}_{p}_{ip}_{s}")
                            nc.tensor.matmul(st2[:, cs0:QB],
                                             kt_t[hs, p, 128 * i0:128 * (i0 + 1)],
                                             q_tile[hs, cs0:QB],
                                             start=True, stop=True, tile_position=tp)
                            nc.tensor.matmul(st2[:, QB + cs1:2 * QB],
                                             kt_t[hs, p, 128 * i1:128 * (i1 + 1)],
                                             q_tile[hs, cs1:QB],
                                             start=True, stop=True, tile_position=tp)
                            e2 = ep.tile([128, 2 * QB], F32R, tag="e",
                                         name=f"e{j}_{p}_{ip}_{s}")
                            if o1 < 0:
                                nc.scalar.activation(e2[:], st2[:], Act.Exp)
                            else:
                                nc.scalar.activation(e2[:, cs0:QB], st2[:, cs0:QB], Act.Exp)
                                nc.scalar.activation(e2[:, QB + cs1:2 * QB],
                                                     st2[:, QB + cs1:2 * QB], Act.Exp)
                                for (oo, base) in ((o0, 0), (o1, QB)):
                                    if oo < 0:
                                        continue
                                    if oo < 3:
                                        z = slice(base + 128 * oo, base + 128 * (oo + 1))
                                        nc.vector.tensor_tensor(e2[:, z], e2[:, z], m1[:],
                                                                op=Alu.mult)
                                    else:
                                        zz = slice(base + 256, base + 384)
                                        nc.vector.tensor_scalar_mul(e2[:, zz], e2[:, zz], 0.0)
                                        z = slice(base + 384, base + QB)
                                        nc.vector.tensor_tensor(e2[:, z], e2[:, z], m1[:],
                                                                op=Alu.mult)
                            nc.tensor.matmul(ctxa[s][:, cs0:QB], va_t[:, i0, h, :],
                                             e2[:, cs0:QB],
                                             start=(ip == nkt_j // 2 - 1), stop=False)
                            nc.tensor.matmul(ctxa[s][:, cs1:QB], va_t[:, i1, h, :],
                                             e2[:, QB + cs1:2 * QB],
                                             start=False, stop=(ip == 0))
                            if pending_norms and ip != nkt_j // 2 - 1:
                                pending_norms.pop(0)()
                    do_norm(1)
                for f in pending_norms:
                    f()
                pending_norms = []

                # output projection for this q-block
                for m in range(8):
                    pf = pca.tile([128, QB], F32, tag="ctxa", name=f"pf{j}_{m}")
                    for p in range(NP):
                        nc.tensor.matmul(pf[:, 0:QB], wp_t[:, p, 128 * m:128 * (m + 1)],
                                         ctx_tiles[(p, j)][:], start=(p == 0), stop=(p == 3))
                    ob = osb.tile([128, QB], F32, tag="ob", name=f"ob{j}_{m}")
                    nc.vector.tensor_copy(ob[:], pf[:, 0:QB])
                    nc.sync.dma_start(outt[128 * m:128 * (m + 1), QB * j:QB * (j + 1)], ob[:])
    return nc


def _split_waits(nc, limit=1):
    """This walrus build accepts only one sync wait per TPB_CTRL instruction;
    move excess waits onto preceding same-engine NOPs."""
    import concourse.mybir as mybir
    for f in nc.m.functions:
        for bb in f.blocks:
            new_insts = []
            for inst in bb.instructions:
                si = inst.sync_info
                if si is not None and si.on_wait and len(si.on_wait) > limit:
                    waits = list(si.on_wait)
                    k = 0
                    while len(waits) - k > limit:
                        chunk = waits[k:k + limit]
                        k += limit
                        nop = mybir.InstNoOp(name=f"{inst.name}_ws{k}")
                        nop.engine = inst.engine
                        nop.sync_info = mybir.SyncInfo(on_wait=chunk, on_update=[])
                        new_insts.append(nop)
                    si.on_wait = waits[k:]
                new_insts.append(inst)
            bb.instructions = new_insts


# --------------------------------------------------------------------------
# compile + SPMD execution via PJRT (axon) — jit once, reuse
# --------------------------------------------------------------------------
class _Compiled:
    def __init__(self, n_cores=8):
        import jax
        from jax.sharding import Mesh, PartitionSpec
        from jax.experimental.shard_map import shard_map
        import concourse.mybir as mybir
        from concourse.bass2jax import (_bass_exec_p, install_neuronx_cc_hook,
                                        partition_id_tensor)

        nc = _build_bass()
        _split_waits(nc)
        install_neuronx_cc_hook()
        partition_name = nc.partition_id_tensor.name if nc.partition_id_tensor else None
        in_names, out_names, out_avals, zero_outs = [], [], [], []
        for alloc in nc.m.functions[0].allocations:
            if not isinstance(alloc, mybir.MemoryLocationSet):
                continue
            name = alloc.memorylocations[0].name
            if alloc.kind == "ExternalInput":
                if name != partition_name:
                    in_names.append(name)
            elif alloc.kind == "ExternalOutput":
                shape = tuple(alloc.tensor_shape)
                dtype = mybir.dt.np(alloc.dtype)
                out_names.append(name)
                out_avals.append(jax.core.ShapedArray(shape, dtype))
                zero_outs.append(np.zeros(shape, dtype))
        n_params = len(in_names)
        all_in_names = list(in_names) + list(out_names)
        if partition_name is not None:
            all_in_names.append(partition_name)

        def _body(*args):
            operands = list(args)
            if partition_name is not None:
                operands.append(partition_id_tensor())
            outs = _bass_exec_p.bind(
                *operands,
                out_avals=tuple(out_avals),
                in_names=tuple(all_in_names),
                out_names=tuple(out_names),
                lowering_input_output_aliases=(),
                sim_require_finite=True,
                sim_require_nnan=True,
                nc=nc,
            )
            return tuple(outs)

        devices = jax.devices()[:n_cores]
        assert len(devices) >= n_cores, f"need {n_cores} cores, have {len(devices)}"
        self.n_cores = n_cores
        self.in_names, self.out_names = in_names, out_names
        self.out_avals, self.zero_outs = out_avals, zero_outs
        mesh = Mesh(np.asarray(devices[:n_cores]), ("core",))
        in_specs = (PartitionSpec("core"),) * (n_params + len(out_names))
        out_specs = (PartitionSpec("core"),) * len(out_names)
        self.fn = jax.jit(
            shard_map(_body, mesh=mesh, in_specs=in_specs,
                      out_specs=out_specs, check_rep=False),
            keep_unused=True)

    def run(self, in_maps):
        import jax
        args = []
        for name in self.in_names:
            args.append(np.concatenate([np.asarray(m[name]) for m in in_maps], axis=0))
        for z in self.zero_outs:
            args.append(np.zeros((self.n_cores * z.shape[0], *z.shape[1:]), z.dtype))
        outs = self.fn(*args)
        jax.block_until_ready(outs)
        res = []
        for c in range(self.n_cores):
            d = {}
            for i, name in enumerate(self.out_names):
                a = np.asarray(outs[i]).reshape(self.n_cores, *self.out_avals[i].shape)[c]
                d[name] = a
            res.append(d)
        return res


# --------------------------------------------------------------------------
# host-side shard / unshard
# --------------------------------------------------------------------------
def _make_core_inputs(x, Wq, Wk, Wv, Wp, core):
    g = core % 2
    b = core // 2
    rows = slice(512 * g, 512 * (g + 1))
    kl = np.arange(128)
    return {
        "xt": np.ascontiguousarray(x[b].T.astype(np.float32)),
        # fold the 1/sqrt(head_dim) score scale into Wq
        "wq": np.ascontiguousarray((Wq[rows, :] * 0.125).T.astype(np.float32)),
        "wk": np.ascontiguousarray(Wk[rows, :].T.astype(np.float32)),
        "wv": np.ascontiguousarray(Wv[rows, :].T.astype(np.float32)),
        "wp": np.ascontiguousarray(Wp[:, rows].T.astype(np.float32)),
        "mask1": (kl[:, None] <= kl[None, :]).astype(np.float32),
        "ones64": np.ones((1, 64), np.float32),
        "vones": np.ones((128, NKT * HL), np.float32),
    }


def kernel(x, Wq, Wk, Wv, Wp):
    """Full-input / full-output causal MHA. x: (4, 2048, 1024) fp32;
    Wq/Wk/Wv/Wp: (1024, 1024) fp32. Returns (4, 2048, 1024) fp32."""
    global _COMPILED
    x = np.asarray(x, dtype=np.float32)
    Wq = np.asarray(Wq, dtype=np.float32)
    Wk = np.asarray(Wk, dtype=np.float32)
    Wv = np.asarray(Wv, dtype=np.float32)
    Wp = np.asarray(Wp, dtype=np.float32)
    assert x.shape == (B, T, D), x.shape

    if _COMPILED is None:
        _COMPILED = _Compiled(8)
    in_maps = [_make_core_inputs(x, Wq, Wk, Wv, Wp, c) for c in range(8)]
    results = _COMPILED.run(in_maps)

    out = np.empty((B, T, D), np.float32)
    for b in range(B):
        acc = results[2 * b]["outt"] + results[2 * b + 1]["outt"]
        out[b] = acc.T
    return out



# revision 10
# speedup vs baseline: 1.0981x; 1.0981x over previous
"""Causal multi-head attention (B=4, T=2048, D=1024, H=16) on 8 Trainium2 cores.

Sharding (data + tensor parallel): core c handles batch b = c//2 and head-group
g = c%2 (8 of the 16 heads). Wq/Wk/Wv are column-sharded by head, Wp is
row-sharded; the two per-batch partial outputs are summed on the host (this
replaces the device all-reduce — the host-side sum is the unshard step).

v2 changes vs the 293us baseline:
  - all matmul operands in bf16 (same PE rate as f32r but no N>=256 floor,
    so masked diagonal tiles narrow to their true width), DVE copies run in
    2-byte fast mode, DMA traffic halves, output returns bf16 partials
  - attention inner loop is ACT(exp)-bound; Q/K/V projections of the NEXT
    t-block and the deferred output projection of the PREVIOUS q-block are
    interleaved into the attention instruction stream as PE filler
  - ascending k-tile order (full tiles first, diagonal last) so attention on
    old k-tiles starts before this block's K/V projections finish
  - normalization: 1/denom per head via DVE reciprocal, one K=2 broadcast
    matmul covers both heads of a pair, and the normalize multiply is a
    single scalar_tensor_tensor reading both PSUM operands directly
  - causal mask multiplies run on the idle GPSIMD (Pool) engine
"""
import numpy as np

T = 2048
D = 1024
B = 4
H = 16
HL = 8            # heads per core
NP = 4            # head pairs per core
QB = 512          # q-block width
NQB = T // QB
NKT = T // 128

_COMPILED = None


# --------------------------------------------------------------------------
# bass kernel build
# --------------------------------------------------------------------------
def _build_bass():
    import concourse.bass as bass
    import concourse.mybir as mybir
    from concourse.tile import TileContext

    F32 = mybir.dt.float32
    F32R = mybir.dt.float32r
    BF16 = mybir.dt.bfloat16
    Act = mybir.ActivationFunctionType
    Alu = mybir.AluOpType

    nc = bass.Bass()
    xt = nc.dram_tensor("xt", [D, T], BF16, kind="ExternalInput")
    wq = nc.dram_tensor("wq", [D, 512], BF16, kind="ExternalInput")
    wk = nc.dram_tensor("wk", [D, 512], BF16, kind="ExternalInput")
    wv = nc.dram_tensor("wv", [D, 512], BF16, kind="ExternalInput")
    wp = nc.dram_tensor("wp", [512, D], BF16, kind="ExternalInput")
    mask1 = nc.dram_tensor("mask1", [128, 128], BF16, kind="ExternalInput")
    on2 = nc.dram_tensor("on2", [2, 128], BF16, kind="ExternalInput")
    outt = nc.dram_tensor("outt", [D, T], BF16, kind="ExternalOutput")

    with TileContext(nc) as tc, nc.allow_low_precision(reason="bf16 pipeline"):
        with tc.tile_pool(name="wts", bufs=1) as wts, \
             tc.tile_pool(name="xp", bufs=2) as xp, \
             tc.tile_pool(name="big", bufs=1) as big, \
             tc.tile_pool(name="qtp", bufs=8) as qtp, \
             tc.tile_pool(name="ctp", bufs=8) as ctp, \
             tc.tile_pool(name="ep", bufs=4) as ep, \
             tc.tile_pool(name="sm", bufs=1) as sm, \
             tc.tile_pool(name="osb", bufs=4) as osb, \
             tc.tile_pool(name="pproj", bufs=2, space="PSUM") as pproj, \
             tc.tile_pool(name="pst", bufs=2, space="PSUM") as pst, \
             tc.tile_pool(name="pca", bufs=2, space="PSUM") as pca:

            # ---------------- weights / constants ----------------
            wq_t = wts.tile([128, 8, 512], BF16, tag="wq")
            wk_t = wts.tile([128, 8, 512], BF16, tag="wk")
            wv_t = wts.tile([128, 8, 512], BF16, tag="wv")
            wp_t = wts.tile([128, 4, 1024], BF16, tag="wp")
            wqr = wq[:].rearrange("(n p) m -> p n m", p=128)
            wkr = wk[:].rearrange("(n p) m -> p n m", p=128)
            wvr = wv[:].rearrange("(n p) m -> p n m", p=128)
            wpr = wp[:].rearrange("(n p) m -> p n m", p=128)

            x_tiles = {}

            def load_x(tb, split=2):
                x_t = xp.tile([128, 8, QB], BF16, tag="x", name=f"x_t{tb}")
                xr = xt[:, QB * tb:QB * (tb + 1)].rearrange(
                    "(n p) m -> p n m", p=128)
                step = 8 // split
                for kk in range(0, 8, step):
                    nc.sync.dma_start(x_t[:, kk:kk + step, :],
                                      xr[:, kk:kk + step, :])
                x_tiles[tb] = x_t

            # startup: small leading slices of wq/x0 so the first Q matmuls
            # start as early as possible, then the remainder in halves
            x_t0 = xp.tile([128, 8, QB], BF16, tag="x", name="x_t0")
            xr0 = xt[:, 0:QB].rearrange("(n p) m -> p n m", p=128)
            nc.sync.dma_start(wq_t[:, 0:2, :], wqr[:, 0:2, :])
            nc.sync.dma_start(x_t0[:, 0:2, :], xr0[:, 0:2, :])
            nc.sync.dma_start(wq_t[:, 2:5, :], wqr[:, 2:5, :])
            nc.sync.dma_start(x_t0[:, 2:5, :], xr0[:, 2:5, :])
            nc.sync.dma_start(wq_t[:, 5:8, :], wqr[:, 5:8, :])
            nc.sync.dma_start(x_t0[:, 5:8, :], xr0[:, 5:8, :])
            x_tiles[0] = x_t0
            nc.sync.dma_start(wk_t[:, 0:4, :], wkr[:, 0:4, :])
            nc.sync.dma_start(wk_t[:, 4:8, :], wkr[:, 4:8, :])
            nc.sync.dma_start(wv_t[:, 0:4, :], wvr[:, 0:4, :])
            nc.sync.dma_start(wv_t[:, 4:8, :], wvr[:, 4:8, :])
            m1 = sm.tile([128, 128], BF16, tag="m1")
            nc.sync.dma_start(m1[:], mask1[:])
            o2 = sm.tile([2, 128], BF16, tag="on2")
            nc.sync.dma_start(o2[:], on2[:])
            for kk in range(4):
                nc.sync.dma_start(wp_t[:, kk, :], wpr[:, kk, :])

            kt_t = big.tile([128, NP, T], BF16, tag="kt")
            va_t = big.tile([128, NKT, HL, 65], BF16, tag="va")
            # ones column for the softmax denominator (row 64 of ctxa)
            nc.gpsimd.memset(va_t[:, :, :, 64:65], 1.0)

            # ---------------- filler machinery ----------------
            # filler work is organized as CHAINS (one PSUM accumulation each,
            # ~4 units of ~2 matmuls); a central dispatcher guarantees at
            # most one chain is mid-flight so the pproj pool (2 bufs) always
            # has a slot for the norm broadcast matmul.
            # due_q/due_kv[tb] hold chains of t-block tb with a deadline
            # inside attention(tb); free_chains can run anywhere.
            free_chains = []
            due_q = {}
            due_kv = {}
            active = []

            def emit_unit(select):
                if not active:
                    ch = select()
                    if ch is None:
                        return False
                    active.extend(ch)
                active.pop(0)()
                return True

            def finish_active():
                while active:
                    active.pop(0)()

            def pop_free(n=1, tb_next=None):
                # never eats due_kv[tb_next]: those are the filler reserve
                # for the next attention block
                def sel():
                    if free_chains:
                        return free_chains.pop(0)[1]
                    if tb_next is not None and due_q.get(tb_next):
                        return due_q[tb_next].pop(0)
                    return None

                for _ in range(n):
                    if not emit_unit(sel):
                        break

            def drain_expired(j):
                # oproj(j-2) chains must be done before ctx(j) tiles reuse
                # their pool slots
                finish_active()
                while free_chains and free_chains[0][0] <= j:
                    for u in free_chains.pop(0)[1]:
                        u()

            def drain_chains(chains):
                finish_active()
                while chains:
                    for u in chains.pop(0):
                        u()

            def drain_kv(tb):
                d = due_kv.get(tb)
                if d:
                    drain_chains(d["v"])
                    for p in (1, 2, 3):
                        drain_chains(d["k"][p])

            qt_tiles = {}
            ctx_tiles = {}

            def make_proj_chain(kind, tb, p):
                """Emit closures for one projection chain (8 matmuls + copy).
                kind: 'q' -> qt tile, 'k' -> kt_t column, 'v' -> va_t tiles.
                """
                st = {"ps": None}
                x_t = x_tiles[tb]

                def unit(k0, kind=kind, tb=tb, p=p):
                    def f():
                        if k0 == 0:
                            st["ps"] = pproj.tile(
                                [128, QB], F32, tag="mm",
                                name=f"ps_{kind}{tb}_{p}")
                        ps = st["ps"]
                        for kk in (k0, k0 + 1):
                            if kind == "v":
                                nc.tensor.matmul(
                                    ps[:], x_t[:, kk, 128 * p:128 * (p + 1)],
                                    wv_t[:, kk, :],
                                    start=(kk == 0), stop=(kk == 7))
                            else:
                                w_t = wq_t if kind == "q" else wk_t
                                nc.tensor.matmul(
                                    ps[:], w_t[:, kk, 128 * p:128 * (p + 1)],
                                    x_t[:, kk, :],
                                    start=(kk == 0), stop=(kk == 7))
                        if k0 == 6:
                            if kind == "q":
                                q_tile = qtp.tile([128, QB], BF16, tag="qt",
                                                  name=f"qt{tb}_{p}")
                                nc.vector.tensor_copy(q_tile[:], ps[:])
                                qt_tiles[(tb, p)] = q_tile
                            elif kind == "k":
                                nc.vector.tensor_copy(
                                    kt_t[:, p, QB * tb:QB * (tb + 1)], ps[:])
                            else:
                                nc.vector.tensor_copy(
                                    va_t[:, 4 * tb + p, :, 0:64],
                                    ps[:].rearrange("p (h d) -> p h d", h=HL))
                    return f

                return [unit(k0) for k0 in range(0, 8, 2)]

            def queue_proj(tb):
                due_q[tb] = [make_proj_chain("q", tb, p) for p in range(NP)]
                # v-class: all V chains + K(p=0) — needed by head-pair 0's
                # diagonal; k-class[p]: K(p) — needed by head-pair p's diagonal
                vs = [make_proj_chain("v", tb, p) for p in range(NP)]
                vs.append(make_proj_chain("k", tb, 0))
                due_kv[tb] = {
                    "v": vs,
                    "k": {p: [make_proj_chain("k", tb, p)] for p in (1, 2, 3)},
                }

            def make_oproj(j):
                """Output projection for q-block j: 8 m-chunks x 4 p-acc."""
                units = []

                def unit(m, phalf, j=j):
                    st_key = ("pf", j, m)

                    def f():
                        if phalf == 0:
                            pf = pproj.tile([128, QB], F32, tag="mm",
                                            name=f"pf{j}_{m}")
                            _oproj_ps[st_key] = pf
                        pf = _oproj_ps[st_key]
                        for p in (phalf * 2, phalf * 2 + 1):
                            nc.tensor.matmul(
                                pf[:], wp_t[:, p, 128 * m:128 * (m + 1)],
                                ctx_tiles[(j, p)][:],
                                start=(p == 0), stop=(p == 3))
                        if phalf == 1:
                            ob = osb.tile([128, QB], BF16, tag="ob",
                                          name=f"ob{j}_{m}")
                            nc.vector.tensor_copy(ob[:], pf[:])
                            nc.sync.dma_start(
                                outt[128 * m:128 * (m + 1),
                                     QB * j:QB * (j + 1)], ob[:])
                    return f

                for m in range(8):
                    units.append([unit(m, 0), unit(m, 1)])
                return units

            _oproj_ps = {}

            # ---------------- attention ----------------
            # norm of a head-pair is deferred into the next head-pair's
            # stream so the PE never waits on the reciprocal chain
            pending_norm = [None]

            def flush_norm():
                if pending_norm[0] is not None:
                    pending_norm[0]()
                    pending_norm[0] = None

            def emit_attention(j):
                nkt_j = 4 * j + 4
                npairs = nkt_j // 2
                # previous block's last norm must land before any filler pop
                # that might read its ctx tiles (deferred output projection)
                flush_norm()
                drain_expired(j)

                def pop_point(p, ip):
                    """One filler pop: v-class scheduled against head-pair
                    0's diagonal deadline, K(p+1) spread across head-pair p,
                    then anything deadline-free."""
                    d = due_kv.get(j)
                    if d and (d["v"] or (active and p == 0)):
                        if p > 0:
                            drain_chains(d["v"])  # deadline passed
                        else:
                            units_left = len(active) + sum(
                                len(c) for c in d["v"])
                            pts_left = max(1, 2 * j - ip)
                            n = (units_left + pts_left - 1) // pts_left
                            pop = lambda: d["v"].pop(0) if d["v"] else None
                            for _ in range(n):
                                if not emit_unit(pop):
                                    break
                        return
                    if d and p < 3 and d["k"].get(p + 1):
                        kl = d["k"][p + 1]
                        emit_unit(lambda: kl.pop(0) if kl else None)
                        return
                    pop_free(1, tb_next=j + 1)

                for p in range(NP):
                    d = due_kv.get(j)
                    if d and d["k"].get(p):
                        drain_chains(d["k"][p])  # K(p) before p's diagonal
                    ctx_tile = ctp.tile([128, QB], BF16, tag="ctx",
                                        name=f"ctx{j}_{p}")
                    ctx_tiles[(j, p)] = ctx_tile
                    q_tile = qt_tiles[(j, p)]
                    ctxa = [pca.tile([65, QB], F32, tag="ctxa",
                                     name=f"ctxa_{j}_{p}_{s2}")
                            for s2 in range(2)]
                    pending = []  # ctx closures, flushed with lag 2
                    for ip in range(npairs):  # ascending k-tiles, s inner
                        pop_point(p, ip)
                        i0, i1 = 2 * ip, 2 * ip + 1
                        o0, o1 = i0 - 4 * j, i1 - 4 * j
                        cs0 = 0 if o0 < 0 else 128 * o0
                        cs1 = 0 if o1 < 0 else 128 * o1
                        for s in range(2):
                            h = 2 * p + s
                            hs = slice(64 * s, 64 * s + 64)
                            tp = (64 * s, 0)
                            st2 = pst.tile([128, 2 * QB], F32, tag="st",
                                           name=f"st{j}_{p}_{s}_{ip}")
                            nc.tensor.matmul(
                                st2[:, cs0:QB],
                                kt_t[hs, p, 128 * i0:128 * (i0 + 1)],
                                q_tile[hs, cs0:QB],
                                start=True, stop=True, tile_position=tp)
                            nc.tensor.matmul(
                                st2[:, QB + cs1:2 * QB],
                                kt_t[hs, p, 128 * i1:128 * (i1 + 1)],
                                q_tile[hs, cs1:QB],
                                start=True, stop=True, tile_position=tp)
                            e2 = ep.tile([128, 2 * QB], BF16, tag="e",
                                         name=f"e{j}_{p}_{s}_{ip}")
                            if o1 < 0:
                                nc.scalar.activation(e2[:], st2[:], Act.Exp)
                            else:
                                nc.scalar.activation(e2[:, cs0:QB],
                                                     st2[:, cs0:QB], Act.Exp)
                                nc.scalar.activation(
                                    e2[:, QB + cs1:2 * QB],
                                    st2[:, QB + cs1:2 * QB], Act.Exp)
                                for (oo, base) in ((o0, 0), (o1, QB)):
                                    if oo < 0:
                                        continue
                                    z = slice(base + 128 * oo,
                                              base + 128 * (oo + 1))
                                    nc.gpsimd.tensor_tensor(
                                        e2[:, z], e2[:, z], m1[:], op=Alu.mult)

                            def ctx_mms(ip=ip, i0=i0, i1=i1, cs0=cs0, cs1=cs1,
                                        e2=e2, s=s, h=h,
                                        last=(ip == npairs - 1)):
                                def f():
                                    nc.tensor.matmul(
                                        ctxa[s][:, cs0:QB], va_t[:, i0, h, :],
                                        e2[:, cs0:QB],
                                        start=(ip == 0), stop=False)
                                    nc.tensor.matmul(
                                        ctxa[s][:, cs1:QB], va_t[:, i1, h, :],
                                        e2[:, QB + cs1:2 * QB],
                                        start=False, stop=last)
                                return f

                            pending.append(ctx_mms())
                        if ip == 1:
                            flush_norm()  # previous head-pair's norm bundle
                        while len(pending) > 4:
                            pending.pop(0)()
                    # flush remaining ctx with the reciprocals interleaved so
                    # the DVE starts on the denominators immediately
                    recip2 = sm.tile([1, 2, QB], BF16, tag="recip", bufs=2,
                                     name=f"recip{j}_{p}")
                    n_tail = len(pending)
                    for i, f in enumerate(pending):
                        f()
                        if i == n_tail - 2:
                            nc.vector.reciprocal(recip2[0:1, 0, :],
                                                 ctxa[0][64:65, :])
                    nc.vector.reciprocal(recip2[0:1, 1, :], ctxa[1][64:65, :])
                    pop_free(2, tb_next=j + 1)

                    def norm_bundle(recip2=recip2, ctxa=ctxa,
                                    ctx_tile=ctx_tile, j=j, p=p):
                        bcp = pproj.tile([128, QB], F32, tag="mm",
                                         name=f"bcp{j}_{p}")
                        nc.tensor.matmul(bcp[0:64, :], o2[0:1, 0:64],
                                         recip2[0:1, 0, :],
                                         start=True, stop=True)
                        nc.tensor.matmul(bcp[64:128, :], o2[0:1, 0:64],
                                         recip2[0:1, 1, :],
                                         start=True, stop=True,
                                         tile_position=(0, 64))
                        # only one non-scalar PSUM operand is allowed per DVE
                        # op, so the broadcast goes through SBUF
                        bcp_sb = sm.tile([128, QB], BF16, tag="bcps", bufs=2,
                                         name=f"bcps{j}_{p}")
                        nc.vector.tensor_copy(bcp_sb[:], bcp[:])
                        for s in range(2):
                            nc.vector.scalar_tensor_tensor(
                                ctx_tile[64 * s:64 * s + 64, :],
                                ctxa[s][0:64, :], 1.0,
                                bcp_sb[64 * s:64 * s + 64, :],
                                op0=Alu.mult, op1=Alu.mult)

                    flush_norm()  # in case npairs < 2 (j=0)
                    pending_norm[0] = norm_bundle

            # ---------------- schedule ----------------
            load_x(1)
            # tb=0: bulk projections (no attention work exists yet)
            queue_proj(0)
            drain_chains(due_q[0])
            drain_kv(0)
            queue_proj(1)
            emit_attention(0)

            for tb in range(1, NQB):
                drain_chains(due_q[tb])  # q tiles needed at head-pair starts
                if tb + 1 < NQB:
                    load_x(tb + 1)
                    queue_proj(tb + 1)
                free_chains.extend((tb + 1, c) for c in make_oproj(tb - 1))
                emit_attention(tb)

            flush_norm()
            finish_active()
            while free_chains:
                for u in free_chains.pop(0)[1]:
                    u()
            drain_chains(make_oproj(NQB - 1))
    return nc


def _split_waits(nc, limit=1):
    """This walrus build accepts only one sync wait per TPB_CTRL instruction;
    move excess waits onto preceding same-engine NOPs."""
    import concourse.mybir as mybir
    for f in nc.m.functions:
        for bb in f.blocks:
            new_insts = []
            for inst in bb.instructions:
                si = inst.sync_info
                if si is not None and si.on_wait and len(si.on_wait) > limit:
                    waits = list(si.on_wait)
                    k = 0
                    while len(waits) - k > limit:
                        chunk = waits[k:k + limit]
                        k += limit
                        nop = mybir.InstNoOp(name=f"{inst.name}_ws{k}")
                        nop.engine = inst.engine
                        nop.sync_info = mybir.SyncInfo(on_wait=chunk, on_update=[])
                        new_insts.append(nop)
                    si.on_wait = waits[k:]
                new_insts.append(inst)
            bb.instructions = new_insts


# --------------------------------------------------------------------------
# compile + SPMD execution via PJRT (axon) — jit once, reuse
# --------------------------------------------------------------------------
class _Compiled:
    def __init__(self, n_cores=8):
        import jax
        from jax.sharding import Mesh, PartitionSpec
        from jax.experimental.shard_map import shard_map
        import concourse.mybir as mybir
        from concourse.bass2jax import (_bass_exec_p, install_neuronx_cc_hook,
                                        partition_id_tensor)

        nc = _build_bass()
        _split_waits(nc)
        install_neuronx_cc_hook()
        partition_name = nc.partition_id_tensor.name if nc.partition_id_tensor else None
        in_names, out_names, out_avals, zero_outs = [], [], [], []
        for alloc in nc.m.functions[0].allocations:
            if not isinstance(alloc, mybir.MemoryLocationSet):
                continue
            name = alloc.memorylocations[0].name
            if alloc.kind == "ExternalInput":
                if name != partition_name:
                    in_names.append(name)
            elif alloc.kind == "ExternalOutput":
                shape = tuple(alloc.tensor_shape)
                dtype = mybir.dt.np(alloc.dtype)
                out_names.append(name)
                out_avals.append(jax.core.ShapedArray(shape, dtype))
                zero_outs.append(np.zeros(shape, dtype))
        n_params = len(in_names)
        all_in_names = list(in_names) + list(out_names)
        if partition_name is not None:
            all_in_names.append(partition_name)

        def _body(*args):
            operands = list(args)
            if partition_name is not None:
                operands.append(partition_id_tensor())
            outs = _bass_exec_p.bind(
                *operands,
                out_avals=tuple(out_avals),
                in_names=tuple(all_in_names),
                out_names=tuple(out_names),
                lowering_input_output_aliases=(),
                sim_require_finite=True,
                sim_require_nnan=True,
                nc=nc,
            )
            return tuple(outs)

        devices = jax.devices()[:n_cores]
        assert len(devices) >= n_cores, f"need {n_cores} cores, have {len(devices)}"
        self.n_cores = n_cores
        self.in_names, self.out_names = in_names, out_names
        self.out_avals, self.zero_outs = out_avals, zero_outs
        mesh = Mesh(np.asarray(devices[:n_cores]), ("core",))
        in_specs = (PartitionSpec("core"),) * (n_params + len(out_names))
        out_specs = (PartitionSpec("core"),) * len(out_names)
        self.fn = jax.jit(
            shard_map(_body, mesh=mesh, in_specs=in_specs,
                      out_specs=out_specs, check_rep=False),
            keep_unused=True)

    def run(self, in_maps):
        import jax
        args = []
        for name in self.in_names:
            args.append(np.concatenate([np.asarray(m[name]) for m in in_maps], axis=0))
        for z in self.zero_outs:
            args.append(np.zeros((self.n_cores * z.shape[0], *z.shape[1:]), z.dtype))
        outs = self.fn(*args)
        jax.block_until_ready(outs)
        res = []
        for c in range(self.n_cores):
            d = {}
            for i, name in enumerate(self.out_names):
                a = np.asarray(outs[i]).reshape(self.n_cores, *self.out_avals[i].shape)[c]
                d[name] = a
            res.append(d)
        return res


# --------------------------------------------------------------------------
# host-side shard / unshard
# --------------------------------------------------------------------------
def _make_core_inputs(x, Wq, Wk, Wv, Wp, core):
    import ml_dtypes
    bf16 = ml_dtypes.bfloat16
    g = core % 2
    b = core // 2
    rows = slice(512 * g, 512 * (g + 1))
    kl = np.arange(128)
    on2 = np.zeros((2, 128), bf16)
    on2[0, 0:64] = 1.0
    on2[1, 64:128] = 1.0
    return {
        "xt": np.ascontiguousarray(x[b].T).astype(bf16),
        # fold the 1/sqrt(head_dim) score scale into Wq
        "wq": np.ascontiguousarray((Wq[rows, :] * 0.125).T).astype(bf16),
        "wk": np.ascontiguousarray(Wk[rows, :].T).astype(bf16),
        "wv": np.ascontiguousarray(Wv[rows, :].T).astype(bf16),
        "wp": np.ascontiguousarray(Wp[:, rows].T).astype(bf16),
        "mask1": (kl[:, None] <= kl[None, :]).astype(bf16),
        "on2": on2,
    }


def kernel(x, Wq, Wk, Wv, Wp):
    """Full-input / full-output causal MHA. x: (4, 2048, 1024) fp32;
    Wq/Wk/Wv/Wp: (1024, 1024) fp32. Returns (4, 2048, 1024) fp32."""
    global _COMPILED
    x = np.asarray(x, dtype=np.float32)
    Wq = np.asarray(Wq, dtype=np.float32)
    Wk = np.asarray(Wk, dtype=np.float32)
    Wv = np.asarray(Wv, dtype=np.float32)
    Wp = np.asarray(Wp, dtype=np.float32)
    assert x.shape == (B, T, D), x.shape

    if _COMPILED is None:
        _COMPILED = _Compiled(8)
    in_maps = [_make_core_inputs(x, Wq, Wk, Wv, Wp, c) for c in range(8)]
    results = _COMPILED.run(in_maps)

    out = np.empty((B, T, D), np.float32)
    for b in range(B):
        acc = (results[2 * b]["outt"].astype(np.float32)
               + results[2 * b + 1]["outt"].astype(np.float32))
        out[b] = acc.T
    return out


# revision 25
# speedup vs baseline: 1.1010x; 1.0026x over previous
"""Causal multi-head attention (B=4, T=2048, D=1024, H=16) on 8 Trainium2 cores.

Sharding (data + tensor parallel): core c handles batch b = c//2 and head-group
g = c%2 (8 of the 16 heads). Wq/Wk/Wv are column-sharded by head, Wp is
row-sharded; the two per-batch partial outputs are summed on the host (this
replaces the device all-reduce — the host-side sum is the unshard step).

v2 changes vs the 293us baseline:
  - all matmul operands in bf16 (same PE rate as f32r but no N>=256 floor,
    so masked diagonal tiles narrow to their true width), DVE copies run in
    2-byte fast mode, DMA traffic halves, output returns bf16 partials
  - attention inner loop is ACT(exp)-bound; Q/K/V projections of the NEXT
    t-block and the deferred output projection of the PREVIOUS q-block are
    interleaved into the attention instruction stream as PE filler
  - ascending k-tile order (full tiles first, diagonal last) so attention on
    old k-tiles starts before this block's K/V projections finish
  - normalization: 1/denom per head via DVE reciprocal, one K=2 broadcast
    matmul covers both heads of a pair, and the normalize multiply is a
    single scalar_tensor_tensor reading both PSUM operands directly
  - causal mask multiplies run on the idle GPSIMD (Pool) engine
"""
import numpy as np

T = 2048
D = 1024
B = 4
H = 16
HL = 8            # heads per core
NP = 4            # head pairs per core
QB = 512          # q-block width
NQB = T // QB
NKT = T // 128

_COMPILED = None


# --------------------------------------------------------------------------
# bass kernel build
# --------------------------------------------------------------------------
def _build_bass():
    import concourse.bass as bass
    import concourse.mybir as mybir
    from concourse.tile import TileContext

    F32 = mybir.dt.float32
    F32R = mybir.dt.float32r
    BF16 = mybir.dt.bfloat16
    Act = mybir.ActivationFunctionType
    Alu = mybir.AluOpType

    nc = bass.Bass()
    xt = nc.dram_tensor("xt", [D, T], BF16, kind="ExternalInput")
    wq = nc.dram_tensor("wq", [D, 512], BF16, kind="ExternalInput")
    wk = nc.dram_tensor("wk", [D, 512], BF16, kind="ExternalInput")
    wv = nc.dram_tensor("wv", [D, 512], BF16, kind="ExternalInput")
    wp = nc.dram_tensor("wp", [512, D], BF16, kind="ExternalInput")
    mask1 = nc.dram_tensor("mask1", [128, 128], BF16, kind="ExternalInput")
    on2 = nc.dram_tensor("on2", [2, 128], BF16, kind="ExternalInput")
    outt = nc.dram_tensor("outt", [D, T], BF16, kind="ExternalOutput")

    with TileContext(nc) as tc, nc.allow_low_precision(reason="bf16 pipeline"):
        with tc.tile_pool(name="wts", bufs=1) as wts, \
             tc.tile_pool(name="xp", bufs=2) as xp, \
             tc.tile_pool(name="big", bufs=1) as big, \
             tc.tile_pool(name="qtp", bufs=8) as qtp, \
             tc.tile_pool(name="ctp", bufs=8) as ctp, \
             tc.tile_pool(name="ep", bufs=6) as ep, \
             tc.tile_pool(name="sm", bufs=1) as sm, \
             tc.tile_pool(name="osb", bufs=4) as osb, \
             tc.tile_pool(name="pproj", bufs=2, space="PSUM") as pproj, \
             tc.tile_pool(name="pst", bufs=2, space="PSUM") as pst, \
             tc.tile_pool(name="pca", bufs=2, space="PSUM") as pca:

            # ---------------- weights / constants ----------------
            wq_t = wts.tile([128, 8, 512], BF16, tag="wq")
            wk_t = wts.tile([128, 8, 512], BF16, tag="wk")
            wv_t = wts.tile([128, 8, 512], BF16, tag="wv")
            wp_t = wts.tile([128, 4, 1024], BF16, tag="wp")
            wqr = wq[:].rearrange("(n p) m -> p n m", p=128)
            wkr = wk[:].rearrange("(n p) m -> p n m", p=128)
            wvr = wv[:].rearrange("(n p) m -> p n m", p=128)
            wpr = wp[:].rearrange("(n p) m -> p n m", p=128)

            x_tiles = {}

            def load_x(tb, split=2):
                x_t = xp.tile([128, 8, QB], BF16, tag="x", name=f"x_t{tb}")
                xr = xt[:, QB * tb:QB * (tb + 1)].rearrange(
                    "(n p) m -> p n m", p=128)
                step = 8 // split
                for kk in range(0, 8, step):
                    nc.sync.dma_start(x_t[:, kk:kk + step, :],
                                      xr[:, kk:kk + step, :])
                x_tiles[tb] = x_t

            # startup: small leading slices of wq/x0 so the first Q matmuls
            # start as early as possible, then the remainder in halves
            x_t0 = xp.tile([128, 8, QB], BF16, tag="x", name="x_t0")
            xr0 = xt[:, 0:QB].rearrange("(n p) m -> p n m", p=128)
            nc.gpsimd.dma_start(wq_t[:, 0:2, :], wqr[:, 0:2, :])
            nc.scalar.dma_start(x_t0[:, 0:2, :], xr0[:, 0:2, :])
            nc.scalar.dma_start(wq_t[:, 2:5, :], wqr[:, 2:5, :])
            nc.sync.dma_start(x_t0[:, 2:5, :], xr0[:, 2:5, :])
            nc.sync.dma_start(wq_t[:, 5:8, :], wqr[:, 5:8, :])
            nc.sync.dma_start(x_t0[:, 5:8, :], xr0[:, 5:8, :])
            x_tiles[0] = x_t0
            nc.sync.dma_start(wk_t[:, 0:4, :], wkr[:, 0:4, :])
            nc.sync.dma_start(wk_t[:, 4:8, :], wkr[:, 4:8, :])
            nc.sync.dma_start(wv_t[:, 0:4, :], wvr[:, 0:4, :])
            nc.sync.dma_start(wv_t[:, 4:8, :], wvr[:, 4:8, :])
            m1 = sm.tile([128, 128], BF16, tag="m1")
            nc.sync.dma_start(m1[:], mask1[:])
            o2 = sm.tile([2, 128], BF16, tag="on2")
            nc.sync.dma_start(o2[:], on2[:])
            for kk in range(4):
                nc.sync.dma_start(wp_t[:, kk, :], wpr[:, kk, :])

            kt_t = big.tile([128, NP, T], BF16, tag="kt")
            va_t = big.tile([128, NKT, HL, 65], BF16, tag="va")
            # ones column for the softmax denominator (row 64 of ctxa)
            nc.gpsimd.memset(va_t[:, :, :, 64:65], 1.0)

            # ---------------- filler machinery ----------------
            # filler work is organized as CHAINS (one PSUM accumulation each,
            # ~4 units of ~2 matmuls); a central dispatcher guarantees at
            # most one chain is mid-flight so the pproj pool (2 bufs) always
            # has a slot for the norm broadcast matmul.
            # due_q/due_kv[tb] hold chains of t-block tb with a deadline
            # inside attention(tb); free_chains can run anywhere.
            free_chains = []
            due_q = {}
            due_kv = {}
            active = []

            def emit_unit(select):
                if not active:
                    ch = select()
                    if ch is None:
                        return False
                    active.extend(ch)
                active.pop(0)()
                return True

            def finish_active():
                while active:
                    active.pop(0)()

            def pop_free(n=1, tb_next=None):
                # may eat due_kv[tb_next] except for the final block, whose
                # K/V chains are the only filler reserve left at that point
                def sel():
                    if free_chains:
                        return free_chains.pop(0)[1]
                    if tb_next is not None and due_q.get(tb_next):
                        return due_q[tb_next].pop(0)
                    if tb_next is not None and tb_next < NQB - 1:
                        d = due_kv.get(tb_next)
                        if d:
                            if d["v"]:
                                return d["v"].pop(0)
                            for pp in (1, 2, 3):
                                if d["k"].get(pp):
                                    return d["k"][pp].pop(0)
                    return None

                for _ in range(n):
                    if not emit_unit(sel):
                        break

            def drain_expired(j):
                # oproj(j-2) chains must be done before ctx(j) tiles reuse
                # their pool slots
                finish_active()
                while free_chains and free_chains[0][0] <= j:
                    for u in free_chains.pop(0)[1]:
                        u()

            def drain_chains(chains):
                finish_active()
                while chains:
                    for u in chains.pop(0):
                        u()

            def drain_kv(tb):
                d = due_kv.get(tb)
                if d:
                    drain_chains(d["v"])
                    for p in (1, 2, 3):
                        drain_chains(d["k"][p])

            qt_tiles = {}
            ctx_tiles = {}

            def make_proj_chain(kind, tb, p):
                """Emit closures for one projection chain (8 matmuls + copy).
                kind: 'q' -> qt tile, 'k' -> kt_t column, 'v' -> va_t tiles.
                """
                st = {"ps": None}
                x_t = x_tiles[tb]

                def unit(k0, kind=kind, tb=tb, p=p):
                    def f():
                        if k0 == 0:
                            st["ps"] = pproj.tile(
                                [128, QB], F32, tag="mm",
                                name=f"ps_{kind}{tb}_{p}")
                        ps = st["ps"]
                        for kk in (k0, k0 + 1):
                            if kind == "v":
                                nc.tensor.matmul(
                                    ps[:], x_t[:, kk, 128 * p:128 * (p + 1)],
                                    wv_t[:, kk, :],
                                    start=(kk == 0), stop=(kk == 7))
                            else:
                                w_t = wq_t if kind == "q" else wk_t
                                nc.tensor.matmul(
                                    ps[:], w_t[:, kk, 128 * p:128 * (p + 1)],
                                    x_t[:, kk, :],
                                    start=(kk == 0), stop=(kk == 7))
                        if k0 == 6:
                            if kind == "q":
                                q_tile = qtp.tile([128, QB], BF16, tag="qt",
                                                  name=f"qt{tb}_{p}")
                                nc.vector.tensor_copy(q_tile[:], ps[:])
                                qt_tiles[(tb, p)] = q_tile
                            elif kind == "k":
                                nc.vector.tensor_copy(
                                    kt_t[:, p, QB * tb:QB * (tb + 1)], ps[:])
                            else:
                                nc.vector.tensor_copy(
                                    va_t[:, 4 * tb + p, :, 0:64],
                                    ps[:].rearrange("p (h d) -> p h d", h=HL))
                    return f

                return [unit(k0) for k0 in range(0, 8, 2)]

            def queue_proj(tb):
                due_q[tb] = [make_proj_chain("q", tb, p) for p in range(NP)]
                # v-class: all V chains + K(p=0) — needed by head-pair 0's
                # diagonal; k-class[p]: K(p) — needed by head-pair p's diagonal
                vs = [make_proj_chain("v", tb, p) for p in range(NP)]
                vs.append(make_proj_chain("k", tb, 0))
                due_kv[tb] = {
                    "v": vs,
                    "k": {p: [make_proj_chain("k", tb, p)] for p in (1, 2, 3)},
                }

            def make_oproj(j):
                """Output projection for q-block j: 8 m-chunks x 4 p-acc."""
                units = []

                def unit(m, phalf, j=j):
                    st_key = ("pf", j, m)

                    def f():
                        if phalf == 0:
                            pf = pproj.tile([128, QB], F32, tag="mm",
                                            name=f"pf{j}_{m}")
                            _oproj_ps[st_key] = pf
                        pf = _oproj_ps[st_key]
                        for p in (phalf * 2, phalf * 2 + 1):
                            nc.tensor.matmul(
                                pf[:], wp_t[:, p, 128 * m:128 * (m + 1)],
                                ctx_tiles[(j, p)][:],
                                start=(p == 0), stop=(p == 3))
                        if phalf == 1:
                            ob = osb.tile([128, QB], BF16, tag="ob",
                                          name=f"ob{j}_{m}")
                            nc.vector.tensor_copy(ob[:], pf[:])
                            nc.sync.dma_start(
                                outt[128 * m:128 * (m + 1),
                                     QB * j:QB * (j + 1)], ob[:])
                    return f

                for m in range(8):
                    units.append([unit(m, 0), unit(m, 1)])
                return units

            _oproj_ps = {}
            ob_part = wts.tile([128, 8, QB], F32, tag="obp")

            def make_oproj_pass1(j):
                chains = []

                def unit(m, step, j=j):
                    key = ("pp1", j, m)

                    def f():
                        if step == 0:
                            pf = pproj.tile([128, QB], F32, tag="mm",
                                            name=f"pp1_{j}_{m}")
                            _oproj_ps[key] = pf
                            for p in (0, 1):
                                nc.tensor.matmul(
                                    pf[:], wp_t[:, p, 128 * m:128 * (m + 1)],
                                    ctx_tiles[(j, p)][:],
                                    start=(p == 0), stop=False)
                        else:
                            pf = _oproj_ps[key]
                            nc.tensor.matmul(
                                pf[:], wp_t[:, 2, 128 * m:128 * (m + 1)],
                                ctx_tiles[(j, 2)][:],
                                start=False, stop=True)
                            nc.vector.tensor_copy(ob_part[:, m, :], pf[:])
                    return f

                for m in range(8):
                    chains.append([unit(m, 0), unit(m, 1)])
                return chains

            # ---------------- attention ----------------
            # norm of a head-pair is deferred into the next head-pair's
            # stream so the PE never waits on the reciprocal chain
            pending_norm = [None]

            def flush_norm():
                if pending_norm[0] is not None:
                    pending_norm[0]()
                    pending_norm[0] = None

            def emit_attention(j):
                nkt_j = 4 * j + 4
                npairs = nkt_j // 2
                # previous block's last norm must land before any filler pop
                # that might read its ctx tiles (deferred output projection)
                flush_norm()
                drain_expired(j)

                def pop_point(p, ip):
                    """One filler pop: v-class scheduled against head-pair
                    0's diagonal deadline, K(p+1) spread across head-pair p,
                    then anything deadline-free."""
                    d = due_kv.get(j)
                    if d and (d["v"] or (active and p == 0)):
                        if p > 0:
                            drain_chains(d["v"])  # deadline passed
                        else:
                            units_left = len(active) + sum(
                                len(c) for c in d["v"])
                            pts_left = max(1, 2 * j - ip)
                            n = (units_left + pts_left - 1) // pts_left
                            pop = lambda: d["v"].pop(0) if d["v"] else None
                            for _ in range(n):
                                if not emit_unit(pop):
                                    break
                        return
                    if d and p < 3 and d["k"].get(p + 1):
                        kl = d["k"][p + 1]
                        emit_unit(lambda: kl.pop(0) if kl else None)
                        return
                    pop_free(1, tb_next=j + 1)

                for p in range(NP):
                    d = due_kv.get(j)
                    if d and d["k"].get(p):
                        drain_chains(d["k"][p])  # K(p) before p's diagonal
                    ctx_tile = ctp.tile([128, QB], BF16, tag="ctx",
                                        name=f"ctx{j}_{p}")
                    ctx_tiles[(j, p)] = ctx_tile
                    q_tile = qt_tiles[(j, p)]
                    ctxa = [pca.tile([65, QB], F32, tag="ctxa",
                                     name=f"ctxa_{j}_{p}_{s2}")
                            for s2 in range(2)]
                    pending = []  # ctx closures, flushed with lag 2
                    for ip in range(npairs):  # ascending k-tiles, s inner
                        pop_point(p, ip)
                        while len(pending) > 4:
                            pending.pop(0)()
                        i0, i1 = 2 * ip, 2 * ip + 1
                        o0, o1 = i0 - 4 * j, i1 - 4 * j
                        cs0 = 0 if o0 < 0 else 128 * o0
                        cs1 = 0 if o1 < 0 else 128 * o1
                        for s in range(2):
                            h = 2 * p + s
                            hs = slice(64 * s, 64 * s + 64)
                            tp = (64 * s, 0)
                            st2 = pst.tile([128, 2 * QB], F32, tag="st",
                                           name=f"st{j}_{p}_{s}_{ip}")
                            nc.tensor.matmul(
                                st2[:, cs0:QB],
                                kt_t[hs, p, 128 * i0:128 * (i0 + 1)],
                                q_tile[hs, cs0:QB],
                                start=True, stop=True, tile_position=tp)
                            nc.tensor.matmul(
                                st2[:, QB + cs1:2 * QB],
                                kt_t[hs, p, 128 * i1:128 * (i1 + 1)],
                                q_tile[hs, cs1:QB],
                                start=True, stop=True, tile_position=tp)
                            e2 = ep.tile([128, 2 * QB], BF16, tag="e",
                                         name=f"e{j}_{p}_{s}_{ip}")
                            if o1 < 0:
                                nc.scalar.activation(e2[:], st2[:], Act.Exp)
                            else:
                                nc.scalar.activation(e2[:, cs0:QB],
                                                     st2[:, cs0:QB], Act.Exp)
                                nc.scalar.activation(
                                    e2[:, QB + cs1:2 * QB],
                                    st2[:, QB + cs1:2 * QB], Act.Exp)
                                for (oo, base) in ((o0, 0), (o1, QB)):
                                    if oo < 0:
                                        continue
                                    z = slice(base + 128 * oo,
                                              base + 128 * (oo + 1))
                                    nc.gpsimd.tensor_tensor(
                                        e2[:, z], e2[:, z], m1[:], op=Alu.mult)

                            def ctx_mms(ip=ip, i0=i0, i1=i1, cs0=cs0, cs1=cs1,
                                        e2=e2, s=s, h=h,
                                        last=(ip == npairs - 1)):
                                def f():
                                    nc.tensor.matmul(
                                        ctxa[s][:, cs0:QB], va_t[:, i0, h, :],
                                        e2[:, cs0:QB],
                                        start=(ip == 0), stop=False)
                                    nc.tensor.matmul(
                                        ctxa[s][:, cs1:QB], va_t[:, i1, h, :],
                                        e2[:, QB + cs1:2 * QB],
                                        start=False, stop=last)
                                return f

                            pending.append(ctx_mms())
                        if ip == 1:
                            flush_norm()  # previous head-pair's norm bundle
                    # flush remaining ctx with the reciprocals interleaved so
                    # the DVE starts on the denominators immediately
                    recip2 = sm.tile([1, 2, QB], BF16, tag="recip", bufs=2,
                                     name=f"recip{j}_{p}")
                    n_tail = len(pending)
                    for i, f in enumerate(pending):
                        f()
                        if i == n_tail - 2:
                            nc.vector.reciprocal(recip2[0:1, 0, :],
                                                 ctxa[0][64:65, :])
                    nc.vector.reciprocal(recip2[0:1, 1, :], ctxa[1][64:65, :])
                    pop_free(2, tb_next=j + 1)

                    def norm_bundle(recip2=recip2, ctxa=ctxa,
                                    ctx_tile=ctx_tile, j=j, p=p):
                        bcp = pproj.tile([128, QB], F32, tag="mm",
                                         name=f"bcp{j}_{p}")
                        nc.tensor.matmul(bcp[0:64, :], o2[0:1, 0:64],
                                         recip2[0:1, 0, :],
                                         start=True, stop=True)
                        nc.tensor.matmul(bcp[64:128, :], o2[0:1, 0:64],
                                         recip2[0:1, 1, :],
                                         start=True, stop=True,
                                         tile_position=(0, 64))
                        # only one non-scalar PSUM operand is allowed per DVE
                        # op, so the broadcast goes through SBUF
                        bcp_sb = sm.tile([128, QB], BF16, tag="bcps", bufs=2,
                                         name=f"bcps{j}_{p}")
                        nc.vector.tensor_copy(bcp_sb[:], bcp[:])
                        for s in range(2):
                            nc.vector.scalar_tensor_tensor(
                                ctx_tile[64 * s:64 * s + 64, :],
                                ctxa[s][0:64, :], 1.0,
                                bcp_sb[64 * s:64 * s + 64, :],
                                op0=Alu.mult, op1=Alu.mult)

                    flush_norm()  # in case npairs < 2 (j=0)
                    pending_norm[0] = norm_bundle

            # ---------------- schedule ----------------
            load_x(1)
            # tb=0: bulk projections (no attention work exists yet)
            queue_proj(0)
            drain_chains(due_q[0])
            drain_kv(0)
            queue_proj(1)
            emit_attention(0)

            for tb in range(1, NQB):
                drain_chains(due_q[tb])  # q tiles needed at head-pair starts
                if tb + 1 < NQB:
                    load_x(tb + 1)
                    queue_proj(tb + 1)
                free_chains.extend((tb + 1, c) for c in make_oproj(tb - 1))
                emit_attention(tb)

            flush_norm()
            finish_active()
            while free_chains:
                for u in free_chains.pop(0)[1]:
                    u()
            drain_chains(make_oproj(NQB - 1))
    return nc


def _split_waits(nc, limit=1):
    """This walrus build accepts only one sync wait per TPB_CTRL instruction;
    move excess waits onto preceding same-engine NOPs."""
    import concourse.mybir as mybir
    for f in nc.m.functions:
        for bb in f.blocks:
            new_insts = []
            for inst in bb.instructions:
                si = inst.sync_info
                if si is not None and si.on_wait and len(si.on_wait) > limit:
                    waits = list(si.on_wait)
                    k = 0
                    while len(waits) - k > limit:
                        chunk = waits[k:k + limit]
                        k += limit
                        nop = mybir.InstNoOp(name=f"{inst.name}_ws{k}")
                        nop.engine = inst.engine
                        nop.sync_info = mybir.SyncInfo(on_wait=chunk, on_update=[])
                        new_insts.append(nop)
                    si.on_wait = waits[k:]
                new_insts.append(inst)
            bb.instructions = new_insts


# --------------------------------------------------------------------------
# compile + SPMD execution via PJRT (axon) — jit once, reuse
# --------------------------------------------------------------------------
class _Compiled:
    def __init__(self, n_cores=8):
        import jax
        from jax.sharding import Mesh, PartitionSpec
        from jax.experimental.shard_map import shard_map
        import concourse.mybir as mybir
        from concourse.bass2jax import (_bass_exec_p, install_neuronx_cc_hook,
                                        partition_id_tensor)

        nc = _build_bass()
        _split_waits(nc)
        install_neuronx_cc_hook()
        partition_name = nc.partition_id_tensor.name if nc.partition_id_tensor else None
        in_names, out_names, out_avals, zero_outs = [], [], [], []
        for alloc in nc.m.functions[0].allocations:
            if not isinstance(alloc, mybir.MemoryLocationSet):
                continue
            name = alloc.memorylocations[0].name
            if alloc.kind == "ExternalInput":
                if name != partition_name:
                    in_names.append(name)
            elif alloc.kind == "ExternalOutput":
                shape = tuple(alloc.tensor_shape)
                dtype = mybir.dt.np(alloc.dtype)
                out_names.append(name)
                out_avals.append(jax.core.ShapedArray(shape, dtype))
                zero_outs.append(np.zeros(shape, dtype))
        n_params = len(in_names)
        all_in_names = list(in_names) + list(out_names)
        if partition_name is not None:
            all_in_names.append(partition_name)

        def _body(*args):
            operands = list(args)
            if partition_name is not None:
                operands.append(partition_id_tensor())
            outs = _bass_exec_p.bind(
                *operands,
                out_avals=tuple(out_avals),
                in_names=tuple(all_in_names),
                out_names=tuple(out_names),
                lowering_input_output_aliases=(),
                sim_require_finite=True,
                sim_require_nnan=True,
                nc=nc,
            )
            return tuple(outs)

        devices = jax.devices()[:n_cores]
        assert len(devices) >= n_cores, f"need {n_cores} cores, have {len(devices)}"
        self.n_cores = n_cores
        self.in_names, self.out_names = in_names, out_names
        self.out_avals, self.zero_outs = out_avals, zero_outs
        mesh = Mesh(np.asarray(devices[:n_cores]), ("core",))
        in_specs = (PartitionSpec("core"),) * (n_params + len(out_names))
        out_specs = (PartitionSpec("core"),) * len(out_names)
        self.fn = jax.jit(
            shard_map(_body, mesh=mesh, in_specs=in_specs,
                      out_specs=out_specs, check_rep=False),
            keep_unused=True)

    def run(self, in_maps):
        import jax
        args = []
        for name in self.in_names:
            args.append(np.concatenate([np.asarray(m[name]) for m in in_maps], axis=0))
        for z in self.zero_outs:
            args.append(np.zeros((self.n_cores * z.shape[0], *z.shape[1:]), z.dtype))
        outs = self.fn(*args)
        jax.block_until_ready(outs)
        res = []
        for c in range(self.n_cores):
            d = {}
            for i, name in enumerate(self.out_names):
                a = np.asarray(outs[i]).reshape(self.n_cores, *self.out_avals[i].shape)[c]
                d[name] = a
            res.append(d)
        return res


# --------------------------------------------------------------------------
# host-side shard / unshard
# --------------------------------------------------------------------------
def _make_core_inputs(x, Wq, Wk, Wv, Wp, core):
    import ml_dtypes
    bf16 = ml_dtypes.bfloat16
    g = core % 2
    b = core // 2
    rows = slice(512 * g, 512 * (g + 1))
    kl = np.arange(128)
    on2 = np.zeros((2, 128), bf16)
    on2[0, 0:64] = 1.0
    on2[1, 64:128] = 1.0
    return {
        "xt": np.ascontiguousarray(x[b].T).astype(bf16),
        # fold the 1/sqrt(head_dim) score scale into Wq
        "wq": np.ascontiguousarray((Wq[rows, :] * 0.125).T).astype(bf16),
        "wk": np.ascontiguousarray(Wk[rows, :].T).astype(bf16),
        "wv": np.ascontiguousarray(Wv[rows, :].T).astype(bf16),
        "wp": np.ascontiguousarray(Wp[:, rows].T).astype(bf16),
        "mask1": (kl[:, None] <= kl[None, :]).astype(bf16),
        "on2": on2,
    }


def kernel(x, Wq, Wk, Wv, Wp):
    """Full-input / full-output causal MHA. x: (4, 2048, 1024) fp32;
    Wq/Wk/Wv/Wp: (1024, 1024) fp32. Returns (4, 2048, 1024) fp32."""
    global _COMPILED
    x = np.asarray(x, dtype=np.float32)
    Wq = np.asarray(Wq, dtype=np.float32)
    Wk = np.asarray(Wk, dtype=np.float32)
    Wv = np.asarray(Wv, dtype=np.float32)
    Wp = np.asarray(Wp, dtype=np.float32)
    assert x.shape == (B, T, D), x.shape

    if _COMPILED is None:
        _COMPILED = _Compiled(8)
    in_maps = [_make_core_inputs(x, Wq, Wk, Wv, Wp, c) for c in range(8)]
    results = _COMPILED.run(in_maps)

    out = np.empty((B, T, D), np.float32)
    for b in range(B):
        acc = (results[2 * b]["outt"].astype(np.float32)
               + results[2 * b + 1]["outt"].astype(np.float32))
        out[b] = acc.T
    return out
